# revision 18
# baseline (speedup 1.0000x reference)
"""Trainium2 Bass kernel for nn_HeatEquation1D.

The reference applies a fixed 62x62 Crank-Nicolson step matrix 100 times to
u0[:, 1:-1] via lax.scan, then zero-pads the boundary columns.  Algebraically
that whole scan is a single matmul:

    out = u0 @ W64,   W64[1:63, 1:63] = (step_matrix^100).T,  zero elsewhere

W64 is computed on the host in float64.  The rel-err budget (2e-2) admits
bf16 (measured rel err ~2.8e-3), so the host:
  - rounds u0 to bf16 and lays it out transposed-interleaved per core:
      ut[k, p] = u[2p + r, f]   where k = r*64 + f  (r in {0,1})
    i.e. partition k holds (row-within-pair r, feature f); this puts the
    contraction dim on partitions using all 128 of them, and device loads
    are plain contiguous DMAs (no DMA-transpose, no PE transpose);
  - upcasts the device's bf16 output back to f32 and un-interleaves.

Device kernel (per core, pure data parallel over 8 cores):
  - 8 plain 1 MiB loads (sync/HWDGE ring) of ut tiles x = [128, 4096].
  - One constant stationary BD = block_diag(W64, W64) (bf16, symmetric);
    matmul(yp, lhsT=BD, rhs=x[:, j:j+512]) gives Y in the same interleaved
    layout: yp[(r, f'), p] = (u @ W64)[2p + r, f'].
  - DVE and ACT alternately copy yp (PSUM f32) -> ys (SBUF bf16).
  - 8 plain 1 MiB stores on the gpsimd/SWDGE queue (separate DMA queue and
    semaphore pool from the loads, so the streams overlap).

Per-core HBM traffic: 2 x 8.39 MB, both streams at plain-DMA full rate.
"""

import numpy as np
import ml_dtypes

BATCH = 524288
NX = 64
NUM_STEPS = 100
N_CORES = 8
ROWS_PER_CORE = BATCH // N_CORES          # 65536
P = 128
PAIRS = ROWS_PER_CORE // 2                 # 32768 row-pairs per core

BLOCK_PAIRS = 4096                         # row-pairs per load/store DMA (1 MiB bf16)
N_BLOCKS = PAIRS // BLOCK_PAIRS            # 8 loads + 8 stores + 1 const = 17 DMAs
MM_N = 512                                 # matmul moving free dim (fp32 out -> 1 bank)

TRACE = False
LAST_RESULTS = None

_NC_CACHE = {}


def _build_nc():
    from concourse import bacc, mybir
    from concourse.tile import TileContext

    nc = bacc.Bacc("TRN2", target_bir_lowering=False, debug=False)
    f32 = mybir.dt.float32
    bf16 = mybir.dt.bfloat16

    ut = nc.dram_tensor("ut", [P, PAIRS], bf16, kind="ExternalInput")
    bd_d = nc.dram_tensor("bd", [P, P], bf16, kind="ExternalInput")
    out = nc.dram_tensor("out", [P, PAIRS], bf16, kind="ExternalOutput")

    ut_r = ut.rearrange("k (nb rp) -> nb k rp", rp=BLOCK_PAIRS)
    out_r = out.rearrange("k (nb rp) -> nb k rp", rp=BLOCK_PAIRS)

    with TileContext(nc) as tc:
        with (
            tc.tile_pool(name="consts", bufs=1) as cpool,
            tc.tile_pool(name="xin", bufs=5) as xpool,
            tc.tile_pool(name="yout", bufs=5) as ypool,
            tc.tile_pool(name="ps_y", bufs=4, space="PSUM") as psy,
        ):
            bd_s = cpool.tile([P, P], bf16)
            nc.gpsimd.dma_start(out=bd_s[:], in_=bd_d[:])

            for nb in range(N_BLOCKS):
                x = xpool.tile([P, BLOCK_PAIRS], bf16)
                nc.sync.dma_start(out=x[:], in_=ut_r[nb])

                ys = ypool.tile([P, BLOCK_PAIRS], bf16)
                for j in range(BLOCK_PAIRS // (2 * MM_N)):
                    yp = psy.tile([P, 2, MM_N], f32)
                    for m in range(2):
                        c0 = (2 * j + m) * MM_N
                        nc.tensor.matmul(
                            yp[:, m],
                            bd_s[:],
                            x[:, c0 : c0 + MM_N],
                            start=True,
                            stop=True,
                        )
                    dst = ys[:, 2 * j * MM_N : 2 * (j + 1) * MM_N]
                    if j % 2 == 0:
                        nc.vector.tensor_copy(out=dst, in_=yp[:])
                    else:
                        nc.scalar.copy(out=dst, in_=yp[:])
                nc.gpsimd.dma_start(out=out_r[nb], in_=ys[:])

    nc.compile()
    return nc


def _host_matrix(step_matrix):
    m = np.asarray(step_matrix, dtype=np.float64)
    w_inner = np.linalg.matrix_power(m, NUM_STEPS).T  # right-multiplier, f64
    w64 = np.zeros((NX, NX), dtype=np.float64)
    w64[1 : NX - 1, 1 : NX - 1] = w_inner
    bd = np.zeros((P, P), dtype=np.float64)
    bd[:NX, :NX] = w64
    bd[NX:, NX:] = w64
    return bd.astype(ml_dtypes.bfloat16)


def kernel(u0, step_matrix):
    global LAST_RESULTS
    from concourse.bass_utils import run_bass_kernel_spmd

    u0 = np.asarray(u0)
    assert u0.shape == (BATCH, NX), u0.shape
    u0_bf = u0.astype(ml_dtypes.bfloat16)

    bd = _host_matrix(step_matrix)

    if "nc" not in _NC_CACHE:
        _NC_CACHE["nc"] = _build_nc()
    nc = _NC_CACHE["nc"]

    in_maps = []
    for c in range(N_CORES):
        uc = u0_bf[c * ROWS_PER_CORE : (c + 1) * ROWS_PER_CORE]
        # ut[k = r*64 + f, p] = u[2p + r, f]
        ut = np.ascontiguousarray(
            uc.reshape(PAIRS, 2, NX).transpose(1, 2, 0).reshape(P, PAIRS)
        )
        in_maps.append({"ut": ut, "bd": bd})
    res = run_bass_kernel_spmd(
        nc, in_maps, core_ids=list(range(N_CORES)), trace=TRACE
    )
    LAST_RESULTS = res

    outs = []
    for r in res.results:
        arr = np.asarray(r["out"])  # [128, 32768] bf16, k=(r,f) interleaved
        y = (
            arr.reshape(2, NX, PAIRS)
            .transpose(2, 0, 1)
            .reshape(ROWS_PER_CORE, NX)
            .astype(np.float32)
        )
        outs.append(y)
    return np.concatenate(outs, axis=0)


# revision 19
# speedup vs baseline: 1.1270x; 1.1270x over previous
"""Trainium2 Bass kernel for nn_HeatEquation1D.

The reference applies a fixed 62x62 Crank-Nicolson step matrix 100 times to
u0[:, 1:-1] via lax.scan, then zero-pads the boundary columns.  Algebraically
that whole scan is a single matmul:

    out = u0 @ W64,   W64[1:63, 1:63] = (step_matrix^100).T,  zero elsewhere

W64 is computed on the host in float64.  The rel-err budget (2e-2) admits
bf16 (measured rel err ~2.8e-3), so the host:
  - rounds u0 to bf16 and lays it out transposed-interleaved per core:
      ut[k, p] = u[2p + r, f]   where k = r*64 + f  (r in {0,1})
    i.e. partition k holds (row-within-pair r, feature f); this puts the
    contraction dim on partitions using all 128 of them, and device loads
    are plain contiguous DMAs (no DMA-transpose, no PE transpose);
  - upcasts the device's bf16 output back to f32 and un-interleaves.

Device kernel (per core, pure data parallel over 8 cores):
  - 8 plain 1 MiB loads (sync/HWDGE ring) of ut tiles x = [128, 4096].
  - One constant stationary BD = block_diag(W64, W64) (bf16, symmetric);
    matmul(yp, lhsT=BD, rhs=x[:, j:j+512]) gives Y in the same interleaved
    layout: yp[(r, f'), p] = (u @ W64)[2p + r, f'].
  - DVE and ACT alternately copy yp (PSUM f32) -> ys (SBUF bf16).
  - 8 plain 1 MiB stores on the gpsimd/SWDGE queue (separate DMA queue and
    semaphore pool from the loads, so the streams overlap).

Per-core HBM traffic: 2 x 8.39 MB, both streams at plain-DMA full rate.
"""

import numpy as np
import ml_dtypes

BATCH = 524288
NX = 64
NUM_STEPS = 100
N_CORES = 8
ROWS_PER_CORE = BATCH // N_CORES          # 65536
P = 128
PAIRS = ROWS_PER_CORE // 2                 # 32768 row-pairs per core

BLOCK_PAIRS = 4096                         # row-pairs per load/store DMA (1 MiB bf16)
N_BLOCKS = PAIRS // BLOCK_PAIRS            # 8 loads + 8 stores + 1 const = 17 DMAs
MM_N = 512                                 # matmul moving free dim (fp32 out -> 1 bank)

TRACE = False
LAST_RESULTS = None

_NC_CACHE = {}


def _build_nc():
    from concourse import bacc, mybir
    from concourse.tile import TileContext

    nc = bacc.Bacc("TRN2", target_bir_lowering=False, debug=False)
    f32 = mybir.dt.float32
    bf16 = mybir.dt.bfloat16

    ut = nc.dram_tensor("ut", [P, PAIRS], bf16, kind="ExternalInput")
    bd_d = nc.dram_tensor("bd", [P, P], bf16, kind="ExternalInput")
    out = nc.dram_tensor("out", [P, PAIRS], bf16, kind="ExternalOutput")

    ut_r = ut.rearrange("k (nb rp) -> nb k rp", rp=BLOCK_PAIRS)
    out_r2 = out.rearrange("k (ns rp) -> ns k rp", rp=BLOCK_PAIRS // 2)

    with TileContext(nc) as tc:
        with (
            tc.tile_pool(name="consts", bufs=1) as cpool,
            tc.tile_pool(name="xin", bufs=5) as xpool,
            tc.tile_pool(name="yout", bufs=5) as ypool,
            tc.tile_pool(name="ps_y", bufs=4, space="PSUM") as psy,
        ):
            bd_s = cpool.tile([P, P], bf16)
            nc.gpsimd.dma_start(out=bd_s[:], in_=bd_d[:])

            for nb in range(N_BLOCKS):
                x = xpool.tile([P, BLOCK_PAIRS], bf16)
                nc.sync.dma_start(out=x[:], in_=ut_r[nb])

                ys = ypool.tile([P, BLOCK_PAIRS], bf16)
                for j in range(BLOCK_PAIRS // (2 * MM_N)):
                    yp = psy.tile([P, 2, MM_N], f32)
                    for m in range(2):
                        c0 = (2 * j + m) * MM_N
                        nc.tensor.matmul(
                            yp[:, m],
                            bd_s[:],
                            x[:, c0 : c0 + MM_N],
                            start=True,
                            stop=True,
                        )
                    dst = ys[:, 2 * j * MM_N : 2 * (j + 1) * MM_N]
                    if j % 2 == 0:
                        nc.vector.tensor_copy(out=dst, in_=yp[:])
                    else:
                        nc.scalar.copy(out=dst, in_=yp[:])
                for sh in range(2):
                    nc.gpsimd.dma_start(
                        out=out_r2[2 * nb + sh],
                        in_=ys[:, sh * (BLOCK_PAIRS // 2) :
                               (sh + 1) * (BLOCK_PAIRS // 2)],
                    )

    nc.compile()
    return nc


def _host_matrix(step_matrix):
    m = np.asarray(step_matrix, dtype=np.float64)
    w_inner = np.linalg.matrix_power(m, NUM_STEPS).T  # right-multiplier, f64
    w64 = np.zeros((NX, NX), dtype=np.float64)
    w64[1 : NX - 1, 1 : NX - 1] = w_inner
    bd = np.zeros((P, P), dtype=np.float64)
    bd[:NX, :NX] = w64
    bd[NX:, NX:] = w64
    return bd.astype(ml_dtypes.bfloat16)


def kernel(u0, step_matrix):
    global LAST_RESULTS
    from concourse.bass_utils import run_bass_kernel_spmd

    u0 = np.asarray(u0)
    assert u0.shape == (BATCH, NX), u0.shape
    u0_bf = u0.astype(ml_dtypes.bfloat16)

    bd = _host_matrix(step_matrix)

    if "nc" not in _NC_CACHE:
        _NC_CACHE["nc"] = _build_nc()
    nc = _NC_CACHE["nc"]

    in_maps = []
    for c in range(N_CORES):
        uc = u0_bf[c * ROWS_PER_CORE : (c + 1) * ROWS_PER_CORE]
        # ut[k = r*64 + f, p] = u[2p + r, f]
        ut = np.ascontiguousarray(
            uc.reshape(PAIRS, 2, NX).transpose(1, 2, 0).reshape(P, PAIRS)
        )
        in_maps.append({"ut": ut, "bd": bd})
    res = run_bass_kernel_spmd(
        nc, in_maps, core_ids=list(range(N_CORES)), trace=TRACE
    )
    LAST_RESULTS = res

    outs = []
    for r in res.results:
        arr = np.asarray(r["out"])  # [128, 32768] bf16, k=(r,f) interleaved
        y = (
            arr.reshape(2, NX, PAIRS)
            .transpose(2, 0, 1)
            .reshape(ROWS_PER_CORE, NX)
            .astype(np.float32)
        )
        outs.append(y)
    return np.concatenate(outs, axis=0)
